# revision 12
# baseline (speedup 1.0000x reference)
"""Trainium2 Bass kernel for nn_Aggregator (Linear -> LayerNorm -> segment mean).

Full inputs in, full output out. Strategy:
  - batch is sorted, so shard rows at segment boundaries across 8 cores
    (each core owns a disjoint range of 2048 segments -> no all-reduce).
  - Host folds LayerNorm mean-centering into W/b:  W'' = W - colmean(W),
    b'' = b - mean(b), so h_c = W''x + b'' is exactly mean-centered and
    ln_w/ln_b commute with the segment mean (applied once per window).
  - Tokens are padded per 128-segment window to a uniform tile count so the
    single SPMD program is identical across cores; padding tokens carry a
    window-local batch id of -1 -> selector row all zero -> inert.
  - bf16 matmul path (fp32 matmul costs 2 PE passes), fp32 PSUM accumulate.
  - Elementwise work is batched over groups of 4 tiles (one PSUM bank) and
    minis over super-groups of 16 tiles to amortize per-op overheads:
      PE : 4x h-mm (psum_h4[:,128t] = xT_t @ W''^T), 4x seg-mm
      DVE: h4 = psum_h4 + b''   (one [128,512] tensor_tensor, bf16 out)
           ssq[128,4] = rowsum(sq4) (one 3D tensor_reduce per group)
           rstd16 = 1/s16          (one reciprocal per 16 tiles)
      ACT: sq4 = Square(h4)        (one [128,512] activation per group)
           s16 = Sqrt(ssq16/128+eps) (one per 16 tiles)
      GPS: sel_t = (iota==bt_local_t)*rstd (window-local ids, bf16-exact)
      PE : psum_seg[128,128] += sel_t^T @ h4[:,128t]  (PSUM-accumulated
           over all tiles of the window)
    Window drain: out = psum_seg/max(cnt,1) * ln_w + ln_b*(cnt>0); counts
    come from a host-side bincount of the (index-only) batch tensor.
"""

import math
import numpy as np

P = 128
D = 128          # IN_DIM == OUT_DIM
NSEG = 16384
NCORES = 8
SEG_PER_CORE = NSEG // NCORES   # 2048
WSEG = 64                       # segments per window
NWIN = SEG_PER_CORE // WSEG     # 32 windows of 64 segments per core
EPS = 1e-5
G = 4                           # tiles per PSUM-batched group
SG = 12                         # tiles per mini super-group
CHUNK = 32                      # tiles per x-chunk DMA ([128,4096] bf16 = 1 MiB)
SEL_ENGINE = "gpsimd"           # or "vector"


def _build_program(TW, nwin, seg_per_core):
    import concourse.tile as tile
    from concourse import bacc, mybir

    f32 = mybir.dt.float32
    bf16 = mybir.dt.bfloat16
    AF = mybir.ActivationFunctionType
    OP = mybir.AluOpType

    assert TW % SG == 0
    NTILES = nwin * TW
    NTOK = NTILES * P

    nc = bacc.Bacc(None, target_bir_lowering=False)
    xt = nc.dram_tensor("xt", [P, NTOK], bf16, kind="ExternalInput")
    # packed f32 consts: lnw | lnb | cw | bt_local
    OLW, OLB = 0, D
    OCW = 2 * D
    OBT = OCW + nwin
    CF = OBT + NTILES
    cstf = nc.dram_tensor("cstf", [P, CF], f32, kind="ExternalInput")
    # packed bf16 consts: wa [128] | iota [64] | ones [128] | b4 [512]
    OWA, OIO = 0, D
    OON = OIO + WSEG
    OB4 = OON + D
    CB = OB4 + G * D
    cstb = nc.dram_tensor("cstb", [P, CB], bf16, kind="ExternalInput")
    outd = nc.dram_tensor("out", [seg_per_core, D], f32, kind="ExternalOutput")

    with tile.TileContext(nc) as tc:
        with (
            tc.tile_pool(name="const", bufs=1) as cpool,
            tc.tile_pool(name="xch", bufs=3) as xpool,
            tc.tile_pool(name="h4", bufs=8) as hpool,
            tc.tile_pool(name="sq16", bufs=3) as sqpool,
            tc.tile_pool(name="sel", bufs=8) as selpool,
            tc.tile_pool(name="mini", bufs=3) as minipool,
            tc.tile_pool(name="outp", bufs=2) as outpool,
            tc.tile_pool(name="ph", bufs=4, space="PSUM") as phpool,
            tc.tile_pool(name="ps", bufs=2, space="PSUM") as pspool,
        ):
            cf_sb = cpool.tile([P, CF], f32, tag="cstf")
            nc.sync.dma_start(cf_sb[:], cstf[:])
            cb_sb = cpool.tile([P, CB], bf16, tag="cstb")
            nc.sync.dma_start(cb_sb[:], cstb[:])
            lnw_sb = cf_sb[:, OLW: OLW + D]
            lnb_sb = cf_sb[:, OLB: OLB + D]
            wa_sb = cb_sb[:, OWA: OWA + D]
            iota_sb = cb_sb[:, OIO: OIO + WSEG]
            ones_row = cb_sb[0:1, OON: OON + D]
            b4_row = cb_sb[0:1, OB4: OB4 + G * D]
            sbias = cpool.tile([P, 1], f32, tag="sbias")
            nc.gpsimd.memset(sbias[:], float(EPS))

            xch = None
            for w in range(nwin):
                psum_seg = pspool.tile([WSEG, D], f32, tag="pseg")
                for sg in range(TW // SG):
                    sg0 = sg * SG
                    ssq16 = minipool.tile([P, SG], f32, tag="ssq")
                    sq16 = sqpool.tile([P, SG * D], bf16, tag="sq16")
                    h4s = []
                    for g in range(SG // G):
                        g0 = sg0 + g * G
                        psum_h4 = phpool.tile([P, G * D], f32, tag="ph")
                        # b'' broadcast into PSUM (rank-1 K=1 matmul)
                        nc.tensor.matmul(
                            psum_h4[:], ones_row, b4_row,
                            start=True, stop=False,
                        )
                        for t in range(G):
                            jj = w * TW + g0 + t
                            if jj % CHUNK == 0:
                                csz = min(CHUNK, NTILES - jj) * P
                                xch = xpool.tile([P, csz], bf16, tag="xch")
                                nc.sync.dma_start(
                                    xch[:], xt[:, jj * P: jj * P + csz])
                            k = (jj % CHUNK) * P
                            nc.tensor.matmul(
                                psum_h4[:, t * D: (t + 1) * D],
                                xch[:, k: k + P], wa_sb,
                                start=False, stop=(t == G - 1),
                            )
                        # square from PSUM (h_c already includes b'')
                        nc.scalar.activation(
                            sq16[:, g * G * D: (g + 1) * G * D],
                            psum_h4[:], AF.Square)
                        # plain PSUM -> SBUF copy, split DVE/ACT
                        h4 = hpool.tile([P, G * D], bf16, tag="h4")
                        if g == 0:
                            nc.vector.tensor_copy(h4[:], psum_h4[:])
                        else:
                            nc.scalar.copy(h4[:], psum_h4[:])
                        h4s.append(h4)
                    nc.vector.tensor_reduce(
                        ssq16[:],
                        sq16[:].rearrange("p (s n) -> p s n", n=D),
                        axis=mybir.AxisListType.X, op=OP.add,
                    )
                    s16 = minipool.tile([P, SG], f32, tag="s16")
                    nc.scalar.activation(
                        s16[:], ssq16[:], AF.Sqrt, scale=1.0 / D, bias=sbias[:])
                    rstd16 = minipool.tile([P, SG], f32, tag="rstd")
                    nc.vector.reciprocal(rstd16[:], s16[:])
                    for g in range(SG // G):
                        g0 = sg0 + g * G
                        for t in range(G):
                            jj = w * TW + g0 + t
                            j = g0 + t
                            sel = selpool.tile([P, WSEG], bf16, tag="sel")
                            nc.vector.tensor_scalar(
                                sel[:], iota_sb,
                                cf_sb[:, OBT + jj: OBT + jj + 1],
                                rstd16[:, j - sg0: j - sg0 + 1],
                                OP.is_equal, OP.mult,
                            )
                            nc.tensor.matmul(
                                psum_seg[:], sel[:],
                                h4s[g][:, t * D: (t + 1) * D],
                                start=(j == 0), stop=(j == TW - 1),
                            )
                # ---- drain window w ----
                cmax = minipool.tile([WSEG, 1], f32, tag="cmax")
                nc.vector.tensor_scalar_max(
                    cmax[:], cf_sb[:WSEG, OCW + w: OCW + w + 1], 1.0)
                recip = minipool.tile([WSEG, 1], f32, tag="recip")
                nc.vector.reciprocal(recip[:], cmax[:])
                ind = minipool.tile([WSEG, 1], f32, tag="ind")
                nc.vector.tensor_scalar_min(
                    ind[:], cf_sb[:WSEG, OCW + w: OCW + w + 1], 1.0)
                out1 = outpool.tile([WSEG, D], f32, tag="out1")
                nc.vector.tensor_scalar(
                    out1[:], psum_seg[:], recip[:], None, OP.mult)
                nc.vector.tensor_tensor(
                    out1[:], out1[:], lnw_sb[:WSEG, :], op=OP.mult)
                out2 = outpool.tile([WSEG, D], f32, tag="out2")
                nc.vector.tensor_scalar(
                    out2[:], lnb_sb[:WSEG, :], ind[:], None, OP.mult)
                nc.vector.tensor_tensor(out1[:], out1[:], out2[:], op=OP.add)
                nc.sync.dma_start(outd[w * WSEG: (w + 1) * WSEG, :], out1[:])
    return nc


def _prepare(x, batch, W, b, ln_w, ln_b, nwin=NWIN, ncores=NCORES):
    """Host-side shard/layout prep. Returns (in_maps, TW)."""
    import ml_dtypes
    bf16 = ml_dtypes.bfloat16

    x = np.asarray(x, dtype=np.float32)
    batch = np.asarray(batch).astype(np.int64)
    W = np.asarray(W, dtype=np.float32)
    b = np.asarray(b, dtype=np.float32)
    ln_w = np.asarray(ln_w, dtype=np.float32)
    ln_b = np.asarray(ln_b, dtype=np.float32)

    nseg = ncores * nwin * WSEG
    Wpp = (W - W.mean(axis=0, keepdims=True)).astype(np.float32)
    bpp = (b - b.mean()).astype(np.float32)

    edges = np.searchsorted(batch, np.arange(0, nseg + 1, WSEG))
    wcounts = np.diff(edges)
    TW = max(1, int(math.ceil(wcounts.max() / P)))
    TW = int(math.ceil(TW / SG)) * SG
    NTILES = nwin * TW
    NTOK = NTILES * P

    OLW, OLB = 0, D
    OCW = 2 * D
    OBT = OCW + nwin
    CF = OBT + NTILES
    OWA, OIO = 0, D
    OON = OIO + WSEG
    OB4 = OON + D
    CB = OB4 + G * D

    xb = x.astype(bf16)
    in_maps = []
    for c in range(ncores):
        xt_np = np.zeros((P, NTOK), bf16)
        bt_np = np.full((NTILES * P,), -1.0, np.float32)
        for w in range(nwin):
            g = c * nwin + w
            s, e = int(edges[g]), int(edges[g + 1])
            n = e - s
            col0 = w * TW * P
            if n:
                xt_np[:, col0: col0 + n] = xb[s:e].T
                bt_np[col0: col0 + n] = (
                    batch[s:e] - (c * nwin + w) * WSEG).astype(np.float32)
        base = c * nwin * WSEG
        rs, re = int(edges[c * nwin]), int(edges[(c + 1) * nwin])
        cnts = np.bincount(
            (batch[rs:re] - base).astype(np.int64), minlength=nwin * WSEG
        ).astype(np.float32)

        cf = np.empty((P, CF), np.float32)
        cf[:, OLW: OLW + D] = ln_w[None, :]
        cf[:, OLB: OLB + D] = ln_b[None, :]
        cf[:, OCW: OCW + nwin] = 0.0
        cf[:WSEG, OCW: OCW + nwin] = cnts.reshape(nwin, WSEG).T
        cf[:, OBT: OBT + NTILES] = np.ascontiguousarray(
            bt_np.reshape(NTILES, P).T)
        cb = np.empty((P, CB), bf16)
        cb[:, OWA: OWA + D] = Wpp.T.astype(bf16)
        cb[:, OIO: OIO + WSEG] = np.arange(WSEG, dtype=np.float32)[None, :]
        cb[:, OON: OON + D] = 1.0
        cb[:, OB4: OB4 + G * D] = np.tile(bpp[None, :], (P, G)).astype(bf16)
        in_maps.append({"xt": xt_np, "cstf": cf, "cstb": cb})
    return in_maps, TW


TRACE = False          # set True (e.g. from test.py) to neuron-profile the run
TRACE_DIR = None
LAST = None            # BassKernelResults of the most recent kernel() call


def kernel(x, batch, W, b, ln_w, ln_b):
    from concourse.bass_utils import run_bass_kernel_spmd

    in_maps, TW = _prepare(x, batch, W, b, ln_w, ln_b)
    nc = _build_program(TW, NWIN, SEG_PER_CORE)
    nc.finalize()
    kw = {}
    if TRACE:
        kw = dict(trace=True, tmpdir=TRACE_DIR)
    res = run_bass_kernel_spmd(nc, in_maps, list(range(NCORES)), **kw)
    global LAST
    LAST = res
    out = np.concatenate(
        [res.results[c]["out"] for c in range(NCORES)], axis=0
    ).astype(np.float32)
    return out


# revision 13
# speedup vs baseline: 1.2551x; 1.2551x over previous
"""Trainium2 Bass kernel for nn_Aggregator (Linear -> LayerNorm -> segment mean).

Full inputs in, full output out. Strategy:
  - batch is sorted, so shard rows at segment boundaries across 8 cores
    (each core owns a disjoint range of 2048 segments -> no all-reduce).
  - Host folds LayerNorm mean-centering into W/b:  W'' = W - colmean(W),
    b'' = b - mean(b), so h_c = W''x + b'' is exactly mean-centered and
    ln_w/ln_b commute with the segment mean (applied once per window).
  - Tokens are padded per 128-segment window to a uniform tile count so the
    single SPMD program is identical across cores; padding tokens carry a
    window-local batch id of -1 -> selector row all zero -> inert.
  - bf16 matmul path (fp32 matmul costs 2 PE passes), fp32 PSUM accumulate.
  - Elementwise work is batched over groups of 4 tiles (one PSUM bank) and
    minis over super-groups of 16 tiles to amortize per-op overheads:
      PE : 4x h-mm (psum_h4[:,128t] = xT_t @ W''^T), 4x seg-mm
      DVE: h4 = psum_h4 + b''   (one [128,512] tensor_tensor, bf16 out)
           ssq[128,4] = rowsum(sq4) (one 3D tensor_reduce per group)
           rstd16 = 1/s16          (one reciprocal per 16 tiles)
      ACT: sq4 = Square(h4)        (one [128,512] activation per group)
           s16 = Sqrt(ssq16/128+eps) (one per 16 tiles)
      GPS: sel_t = (iota==bt_local_t)*rstd (window-local ids, bf16-exact)
      PE : psum_seg[128,128] += sel_t^T @ h4[:,128t]  (PSUM-accumulated
           over all tiles of the window)
    Window drain: out = psum_seg/max(cnt,1) * ln_w + ln_b*(cnt>0); counts
    come from a host-side bincount of the (index-only) batch tensor.
"""

import math
import numpy as np

P = 128
D = 128          # IN_DIM == OUT_DIM
NSEG = 16384
NCORES = 8
SEG_PER_CORE = NSEG // NCORES   # 2048
WSEG = 128                      # segments per window
NWIN = SEG_PER_CORE // WSEG     # 16 windows of 128 segments per core
EPS = 1e-5
G = 4                           # tiles per PSUM-batched group
SG = 16                         # tiles per mini super-group
CHUNK = 32                      # tiles per x-chunk DMA ([128,4096] bf16 = 1 MiB)
SEL_ENGINE = "gpsimd"           # or "vector"


def _build_program(TW, nwin, seg_per_core):
    import concourse.tile as tile
    from concourse import bacc, mybir

    f32 = mybir.dt.float32
    bf16 = mybir.dt.bfloat16
    AF = mybir.ActivationFunctionType
    OP = mybir.AluOpType

    assert TW % SG == 0
    NTILES = nwin * TW
    NTOK = NTILES * P

    nc = bacc.Bacc(None, target_bir_lowering=False)
    xt = nc.dram_tensor("xt", [P, NTOK], bf16, kind="ExternalInput")
    # packed f32 consts: lnw | lnb | cw | bt_local
    OLW, OLB = 0, D
    OCW = 2 * D
    OBT = OCW + nwin
    CF = OBT + NTILES
    cstf = nc.dram_tensor("cstf", [P, CF], f32, kind="ExternalInput")
    # packed bf16 consts: wa [128] | iota [64] | ones [128] | b4 [512]
    OWA, OIO = 0, D
    OON = OIO + WSEG
    OB4 = OON + D
    CB = OB4 + G * D
    cstb = nc.dram_tensor("cstb", [P, CB], bf16, kind="ExternalInput")
    outd = nc.dram_tensor("out", [seg_per_core, D], f32, kind="ExternalOutput")

    with tile.TileContext(nc) as tc:
        with (
            tc.tile_pool(name="const", bufs=1) as cpool,
            tc.tile_pool(name="xch", bufs=3) as xpool,
            tc.tile_pool(name="h4", bufs=8) as hpool,
            tc.tile_pool(name="sq16", bufs=3) as sqpool,
            tc.tile_pool(name="sel", bufs=8) as selpool,
            tc.tile_pool(name="mini", bufs=3) as minipool,
            tc.tile_pool(name="outp", bufs=2) as outpool,
            tc.tile_pool(name="ph", bufs=4, space="PSUM") as phpool,
            tc.tile_pool(name="ps", bufs=2, space="PSUM") as pspool,
        ):
            cf_sb = cpool.tile([P, CF], f32, tag="cstf")
            nc.sync.dma_start(cf_sb[:], cstf[:])
            cb_sb = cpool.tile([P, CB], bf16, tag="cstb")
            nc.sync.dma_start(cb_sb[:], cstb[:])
            lnw_sb = cf_sb[:, OLW: OLW + D]
            lnb_sb = cf_sb[:, OLB: OLB + D]
            wa_sb = cb_sb[:, OWA: OWA + D]
            iota_sb = cb_sb[:, OIO: OIO + WSEG]
            ones_row = cb_sb[0:1, OON: OON + D]
            b4_row = cb_sb[0:1, OB4: OB4 + G * D]
            sbias = cpool.tile([P, 1], f32, tag="sbias")
            nc.gpsimd.memset(sbias[:], float(EPS))

            xch = None
            for w in range(nwin):
                psum_seg = pspool.tile([WSEG, D], f32, tag="pseg")
                for sg in range(TW // SG):
                    sg0 = sg * SG
                    ssq16 = minipool.tile([P, SG], f32, tag="ssq")
                    sq16 = sqpool.tile([P, SG * D], bf16, tag="sq16")
                    h4s = []
                    for g in range(SG // G):
                        g0 = sg0 + g * G
                        psum_h4 = phpool.tile([P, G * D], f32, tag="ph")
                        # b'' broadcast into PSUM (rank-1 K=1 matmul)
                        nc.tensor.matmul(
                            psum_h4[:], ones_row, b4_row,
                            start=True, stop=False,
                        )
                        for t in range(G):
                            jj = w * TW + g0 + t
                            if jj % CHUNK == 0:
                                csz = min(CHUNK, NTILES - jj) * P
                                xch = xpool.tile([P, csz], bf16, tag="xch")
                                nc.sync.dma_start(
                                    xch[:], xt[:, jj * P: jj * P + csz])
                            k = (jj % CHUNK) * P
                            nc.tensor.matmul(
                                psum_h4[:, t * D: (t + 1) * D],
                                xch[:, k: k + P], wa_sb,
                                start=False, stop=(t == G - 1),
                            )
                        # square from PSUM (h_c already includes b'')
                        nc.scalar.activation(
                            sq16[:, g * G * D: (g + 1) * G * D],
                            psum_h4[:], AF.Square)
                        # plain PSUM -> SBUF copy, split DVE/ACT
                        h4 = hpool.tile([P, G * D], bf16, tag="h4")
                        nc.scalar.copy(h4[:], psum_h4[:])
                        h4s.append(h4)
                    nc.vector.tensor_reduce(
                        ssq16[:],
                        sq16[:].rearrange("p (s n) -> p s n", n=D),
                        axis=mybir.AxisListType.X, op=OP.add,
                    )
                    s16 = minipool.tile([P, SG], f32, tag="s16")
                    nc.scalar.activation(
                        s16[:], ssq16[:], AF.Sqrt, scale=1.0 / D, bias=sbias[:])
                    rstd16 = minipool.tile([P, SG], f32, tag="rstd")
                    nc.vector.reciprocal(rstd16[:], s16[:])
                    for g in range(SG // G):
                        g0 = sg0 + g * G
                        for t in range(G):
                            jj = w * TW + g0 + t
                            j = g0 + t
                            sel = selpool.tile([P, WSEG], bf16, tag="sel")
                            nc.vector.tensor_scalar(
                                sel[:], iota_sb,
                                cf_sb[:, OBT + jj: OBT + jj + 1],
                                rstd16[:, j - sg0: j - sg0 + 1],
                                OP.is_equal, OP.mult,
                            )
                            nc.tensor.matmul(
                                psum_seg[:], sel[:],
                                h4s[g][:, t * D: (t + 1) * D],
                                start=(j == 0), stop=(j == TW - 1),
                            )
                # ---- drain window w ----
                cmax = minipool.tile([WSEG, 1], f32, tag="cmax")
                nc.vector.tensor_scalar_max(
                    cmax[:], cf_sb[:WSEG, OCW + w: OCW + w + 1], 1.0)
                recip = minipool.tile([WSEG, 1], f32, tag="recip")
                nc.vector.reciprocal(recip[:], cmax[:])
                ind = minipool.tile([WSEG, 1], f32, tag="ind")
                nc.vector.tensor_scalar_min(
                    ind[:], cf_sb[:WSEG, OCW + w: OCW + w + 1], 1.0)
                out1 = outpool.tile([WSEG, D], f32, tag="out1")
                nc.vector.tensor_scalar(
                    out1[:], psum_seg[:], recip[:], None, OP.mult)
                nc.vector.tensor_tensor(
                    out1[:], out1[:], lnw_sb[:WSEG, :], op=OP.mult)
                out2 = outpool.tile([WSEG, D], f32, tag="out2")
                nc.vector.tensor_scalar(
                    out2[:], lnb_sb[:WSEG, :], ind[:], None, OP.mult)
                nc.vector.tensor_tensor(out1[:], out1[:], out2[:], op=OP.add)
                nc.sync.dma_start(outd[w * WSEG: (w + 1) * WSEG, :], out1[:])
    return nc


def _prepare(x, batch, W, b, ln_w, ln_b, nwin=NWIN, ncores=NCORES):
    """Host-side shard/layout prep. Returns (in_maps, TW)."""
    import ml_dtypes
    bf16 = ml_dtypes.bfloat16

    x = np.asarray(x, dtype=np.float32)
    batch = np.asarray(batch).astype(np.int64)
    W = np.asarray(W, dtype=np.float32)
    b = np.asarray(b, dtype=np.float32)
    ln_w = np.asarray(ln_w, dtype=np.float32)
    ln_b = np.asarray(ln_b, dtype=np.float32)

    nseg = ncores * nwin * WSEG
    Wpp = (W - W.mean(axis=0, keepdims=True)).astype(np.float32)
    bpp = (b - b.mean()).astype(np.float32)

    edges = np.searchsorted(batch, np.arange(0, nseg + 1, WSEG))
    wcounts = np.diff(edges)
    TW = max(1, int(math.ceil(wcounts.max() / P)))
    TW = int(math.ceil(TW / SG)) * SG
    NTILES = nwin * TW
    NTOK = NTILES * P

    OLW, OLB = 0, D
    OCW = 2 * D
    OBT = OCW + nwin
    CF = OBT + NTILES
    OWA, OIO = 0, D
    OON = OIO + WSEG
    OB4 = OON + D
    CB = OB4 + G * D

    xb = x.astype(bf16)
    in_maps = []
    for c in range(ncores):
        xt_np = np.zeros((P, NTOK), bf16)
        bt_np = np.full((NTILES * P,), -1.0, np.float32)
        for w in range(nwin):
            g = c * nwin + w
            s, e = int(edges[g]), int(edges[g + 1])
            n = e - s
            col0 = w * TW * P
            if n:
                xt_np[:, col0: col0 + n] = xb[s:e].T
                bt_np[col0: col0 + n] = (
                    batch[s:e] - (c * nwin + w) * WSEG).astype(np.float32)
        base = c * nwin * WSEG
        rs, re = int(edges[c * nwin]), int(edges[(c + 1) * nwin])
        cnts = np.bincount(
            (batch[rs:re] - base).astype(np.int64), minlength=nwin * WSEG
        ).astype(np.float32)

        cf = np.empty((P, CF), np.float32)
        cf[:, OLW: OLW + D] = ln_w[None, :]
        cf[:, OLB: OLB + D] = ln_b[None, :]
        cf[:, OCW: OCW + nwin] = 0.0
        cf[:WSEG, OCW: OCW + nwin] = cnts.reshape(nwin, WSEG).T
        cf[:, OBT: OBT + NTILES] = np.ascontiguousarray(
            bt_np.reshape(NTILES, P).T)
        cb = np.empty((P, CB), bf16)
        cb[:, OWA: OWA + D] = Wpp.T.astype(bf16)
        cb[:, OIO: OIO + WSEG] = np.arange(WSEG, dtype=np.float32)[None, :]
        cb[:, OON: OON + D] = 1.0
        cb[:, OB4: OB4 + G * D] = np.tile(bpp[None, :], (P, G)).astype(bf16)
        in_maps.append({"xt": xt_np, "cstf": cf, "cstb": cb})
    return in_maps, TW


TRACE = False          # set True (e.g. from test.py) to neuron-profile the run
TRACE_DIR = None
LAST = None            # BassKernelResults of the most recent kernel() call


def kernel(x, batch, W, b, ln_w, ln_b):
    from concourse.bass_utils import run_bass_kernel_spmd

    in_maps, TW = _prepare(x, batch, W, b, ln_w, ln_b)
    nc = _build_program(TW, NWIN, SEG_PER_CORE)
    nc.finalize()
    kw = {}
    if TRACE:
        kw = dict(trace=True, tmpdir=TRACE_DIR)
    res = run_bass_kernel_spmd(nc, in_maps, list(range(NCORES)), **kw)
    global LAST
    LAST = res
    out = np.concatenate(
        [res.results[c]["out"] for c in range(NCORES)], axis=0
    ).astype(np.float32)
    return out
